# revision 4
# baseline (speedup 1.0000x reference)
"""MoE layer (8 experts, top-2) for 8 Trainium2 NeuronCores.

Strategy: expert-parallel. The router (0.1% of FLOPs) runs on host and
decides the sharding: tokens are all-to-all'd by routed expert (host-side
gather, since kernel() holds the full inputs). Each core runs one expert's
dense MLP  y = scale * (gelu(x @ W1 + b1) @ W2 + b2)  over the tokens routed
to it, with fp32r (TF32-like full-rate) matmuls on the tensor engine.
Host scatter-adds the per-expert partial outputs back (the unshard).
"""

import os

import numpy as np

HIDDEN = 1024
FF = 2 * HIDDEN
NUM_EXPERTS = 8
TOP_K = 2
NCORES = 8

# Set by kernel() when MOE_TRACE=1: HW kernel execution time in ns.
LAST_EXEC_NS = None
LAST_RESULTS = None

_PROGRAM_CACHE = {}


def _round_up(v, m):
    return (v + m - 1) // m * m


def _build_program(C, blk):
    """Bass/Tile program for one expert MLP over C tokens (SPMD on 8 cores).

    Layouts (per core):
      xT  [HIDDEN, C] f32r  - gathered tokens, transposed
      w1  [HIDDEN, FF] f32r, b1 [FF] f32
      w2  [FF, HIDDEN] f32r, b2 [HIDDEN] f32
      scl [C] f32           - per-token combine weight
      yT  [HIDDEN, C] f32   - output, transposed

    Stage B: hT[f, t] = gelu(sum_h w1[h, f] * xT[h, t] + b1[f])  (PSUM acc over
    8 h-chunks; lhsT = w1 chunk [128h, 128f], rhs = xT chunk [128h, blk])
    Stage C: yT[o, t] = (sum_f w2[f, o] * hT[f, t] + b2[o]) * scl[t]
    """
    import concourse.bass as bass  # noqa: F401
    import concourse.mybir as mybir
    import concourse.tile as tile
    from concourse import bacc

    HC = HIDDEN // 128  # 8 h-chunks
    FC = FF // 128  # 16 f-chunks
    f32 = mybir.dt.float32
    f32r = mybir.dt.float32r

    nc = bacc.Bacc("TRN2", target_bir_lowering=False, debug=False,
                   num_devices=NCORES)
    xT = nc.dram_tensor("xT", [HIDDEN, C], f32r, kind="ExternalInput")
    w1 = nc.dram_tensor("w1", [HIDDEN, FF], f32r, kind="ExternalInput")
    b1 = nc.dram_tensor("b1", [FF], f32, kind="ExternalInput")
    w2 = nc.dram_tensor("w2", [FF, HIDDEN], f32r, kind="ExternalInput")
    b2 = nc.dram_tensor("b2", [HIDDEN], f32, kind="ExternalInput")
    scl = nc.dram_tensor("scl", [C], f32, kind="ExternalInput")
    yT = nc.dram_tensor("yT", [HIDDEN, C], f32, kind="ExternalOutput")

    blocks = []
    t0 = 0
    while t0 < C:
        b = min(blk, C - t0)
        blocks.append((t0, b))
        t0 += b

    Gelu = mybir.ActivationFunctionType.Gelu
    Ident = mybir.ActivationFunctionType.Identity

    with tile.TileContext(nc) as tc:
        with (
            tc.tile_pool(name="wts", bufs=1) as wts,
            tc.tile_pool(name="xin", bufs=2) as xin,
            tc.tile_pool(name="hmid", bufs=1) as hmid,
            tc.tile_pool(name="outs", bufs=3) as outs,
            tc.tile_pool(name="ps", bufs=4, space="PSUM") as ps,
        ):
            # --- resident weights/biases ---
            w1_sb = wts.tile([128, HC, FF], f32r)
            nc.sync.dma_start(
                out=w1_sb[:], in_=w1.ap().rearrange("(c p) f -> p c f", p=128))
            w2_sb = wts.tile([128, FC, HIDDEN], f32r)
            nc.sync.dma_start(
                out=w2_sb[:], in_=w2.ap().rearrange("(c p) h -> p c h", p=128))
            b1_sb = wts.tile([128, FC], f32)
            nc.sync.dma_start(
                out=b1_sb[:], in_=b1.ap().rearrange("(c p) -> p c", p=128))
            b2_sb = wts.tile([128, HC], f32)
            nc.sync.dma_start(
                out=b2_sb[:], in_=b2.ap().rearrange("(c p) -> p c", p=128))

            for t0, bs in blocks:
                x_sb = xin.tile([128, HC, bs], f32r, tag="x")
                nc.sync.dma_start(
                    out=x_sb[:],
                    in_=xT.ap().rearrange("(c p) t -> p c t", p=128)[:, :, t0:t0 + bs])
                s_sb = xin.tile([128, bs], f32, tag="s")
                nc.sync.dma_start(
                    out=s_sb[:], in_=scl.ap()[t0:t0 + bs].partition_broadcast(128))

                h_sb = hmid.tile([128, FC, blk], f32r, tag="h")
                for fc in range(FC):
                    ph = ps.tile([128, blk], f32, tag="ps")
                    for hc in range(HC):
                        nc.tensor.matmul(
                            ph[:, :bs],
                            w1_sb[:, hc, fc * 128:(fc + 1) * 128],
                            x_sb[:, hc, :],
                            start=(hc == 0), stop=(hc == HC - 1),
                        )
                    nc.scalar.activation(
                        out=h_sb[:, fc, :bs], in_=ph[:, :bs],
                        func=Gelu, bias=b1_sb[:, fc:fc + 1], scale=1.0)

                for oc in range(HC):
                    py = ps.tile([128, blk], f32, tag="ps")
                    for fc in range(FC):
                        nc.tensor.matmul(
                            py[:, :bs],
                            w2_sb[:, fc, oc * 128:(oc + 1) * 128],
                            h_sb[:, fc, :bs],
                            start=(fc == 0), stop=(fc == FC - 1),
                        )
                    o1 = outs.tile([128, blk], f32, tag="o1")
                    nc.scalar.activation(
                        out=o1[:, :bs], in_=py[:, :bs], func=Ident,
                        bias=b2_sb[:, oc:oc + 1], scale=1.0)
                    o2 = outs.tile([128, blk], f32, tag="o2")
                    nc.vector.tensor_mul(o2[:, :bs], o1[:, :bs], s_sb[:])
                    nc.sync.dma_start(
                        out=yT.ap().rearrange(
                            "(c p) t -> p c t", p=128)[:, oc, t0:t0 + bs],
                        in_=o2[:, :bs])

    nc.compile()
    return nc


def _route_host(x, Wr, br):
    """Replicate the reference router bit-exactly (jax on CPU)."""
    import jax
    import jax.numpy as jnp

    cpu = jax.devices("cpu")[0]
    xj = jax.device_put(x, cpu)
    Wrj = jax.device_put(Wr, cpu)
    brj = jax.device_put(br, cpu)
    with jax.default_device(cpu):
        logits = jnp.einsum("bsh,he->bse", xj, Wrj) + brj
        routing = jax.nn.softmax(logits, axis=-1)
        topw, topi = jax.lax.top_k(routing, TOP_K)
        topw = jax.nn.softmax(topw, axis=-1)
    return np.asarray(topw), np.asarray(topi)


def kernel(x, Wr, br, W1, b1, W2, b2):
    global LAST_EXEC_NS, LAST_RESULTS
    from concourse.bass_utils import run_bass_kernel_spmd

    x = np.ascontiguousarray(np.asarray(x, dtype=np.float32))
    Wr = np.asarray(Wr, dtype=np.float32)
    br = np.asarray(br, dtype=np.float32)
    W1 = np.ascontiguousarray(np.asarray(W1, dtype=np.float32))
    b1 = np.ascontiguousarray(np.asarray(b1, dtype=np.float32))
    W2 = np.ascontiguousarray(np.asarray(W2, dtype=np.float32))
    b2 = np.ascontiguousarray(np.asarray(b2, dtype=np.float32))

    B, S, H = x.shape
    ntok = B * S
    xf = x.reshape(ntok, H)

    topw, topi = _route_host(x, Wr, br)
    topw = topw.reshape(ntok, TOP_K)
    topi = topi.reshape(ntok, TOP_K)

    # per-expert token index lists + combine weights
    idx = []
    wgt = []
    for e in range(NUM_EXPERTS):
        mask = (topi == e)
        tok = np.nonzero(mask.any(axis=1))[0]
        w = (topw * mask).sum(axis=1)[tok].astype(np.float32)
        idx.append(tok)
        wgt.append(w)
    counts = np.array([len(t) for t in idx])

    blk = int(os.environ.get("MOE_BLK", "256"))
    C = max(_round_up(int(counts.max()), 128), blk)

    key = (C, blk)
    if key not in _PROGRAM_CACHE:
        _PROGRAM_CACHE[key] = _build_program(C, blk)
    nc = _PROGRAM_CACHE[key]

    in_maps = []
    for e in range(NUM_EXPERTS):
        xTe = np.zeros((H, C), dtype=np.float32)
        xTe[:, :counts[e]] = xf[idx[e]].T
        scle = np.zeros((C,), dtype=np.float32)
        scle[:counts[e]] = wgt[e]
        in_maps.append({
            "xT": xTe,
            "w1": np.ascontiguousarray(W1[e]),
            "b1": np.ascontiguousarray(b1[e]),
            "w2": np.ascontiguousarray(W2[e]),
            "b2": np.ascontiguousarray(b2[e]),
            "scl": scle,
        })

    trace = os.environ.get("MOE_TRACE", "0") == "1"
    res = run_bass_kernel_spmd(
        nc, in_maps, core_ids=list(range(NCORES)), trace=trace)
    LAST_EXEC_NS = res.exec_time_ns
    LAST_RESULTS = res

    out = np.zeros((ntok, H), dtype=np.float32)
    for e in range(NUM_EXPERTS):
        ye = res.results[e]["yT"][:, :counts[e]].T  # [cnt, H]
        out[idx[e]] += ye * 1.0
    return out.reshape(B, S, H)
